# revision 1
# baseline (speedup 1.0000x reference)
"""Fused LSTM cell on 8 Trainium2 NeuronCores.

Data-parallel over the batch: each core handles 1024 of the 8192 rows.
Per core, the two GEMMs (x @ Wx.T + h @ Wh.T) are fused into one
[2048]-contraction GEMM in float32r (full-rate fp32 PE mode), with the
gate nonlinearities + state update fused into the PSUM eviction path.

Layouts are chosen so every DMA is a plain strided copy (no on-chip
transposes): activations and weights are pre-transposed on the host so
the contraction dim lands on SBUF partitions, and the whole kernel runs
in [hidden, batch] layout; the host transposes the outputs back.
"""

import os

import numpy as np

import concourse.bacc as bacc
import concourse.mybir as mybir
import concourse.tile as tile
from concourse.bass_utils import run_bass_kernel_spmd

B, I, H = 8192, 1024, 1024
NCORES = 8
BL = B // NCORES        # batch rows per core
G4 = 4 * H              # stacked gate dim
KC = (I + H) // 128     # contraction chunks of 128
HT = H // 128           # h-tiles per core
NBC = 2                 # batch chunks per h-tile
BCW = BL // NBC         # 512 columns per matmul (one PSUM bank)

F32 = mybir.dt.float32
F32R = mybir.dt.float32r
AF = mybir.ActivationFunctionType
OP = mybir.AluOpType

_CACHE: dict = {}


def _build(reps=1, hw_loop=False, variant="full"):
    mm_dt = mybir.dt.bfloat16 if variant == "mm_bf16" else F32R
    nc = bacc.Bacc("TRN2", target_bir_lowering=False, debug=False)
    aT = nc.dram_tensor("a_t", [I + H, BL], mm_dt, kind="ExternalInput")
    wT = nc.dram_tensor("w_t", [I + H, G4], mm_dt, kind="ExternalInput")
    cT = nc.dram_tensor("c_t", [H, BL], F32, kind="ExternalInput")
    bias = nc.dram_tensor("bias", [128, 4 * HT], F32, kind="ExternalInput")
    cO = nc.dram_tensor("c_out", [H, BL], F32, kind="ExternalOutput")
    hO = nc.dram_tensor("h_out", [H, BL], F32, kind="ExternalOutput")

    with tile.TileContext(nc) as tc:
        with (
            tc.tile_pool(name="resident", bufs=1) as res_pool,
            tc.tile_pool(name="wpool", bufs=2) as w_pool,
            tc.tile_pool(name="cpool", bufs=2) as c_pool,
            tc.tile_pool(name="opool", bufs=2) as o_pool,
            tc.tile_pool(name="act", bufs=3) as act_pool,
            tc.tile_pool(name="psum", bufs=2, space="PSUM") as psum_pool,
        ):
            # Activations resident for the whole kernel: [128, 16, 1024]
            a_sb = res_pool.tile([128, KC, BL], mm_dt)
            nc.sync.dma_start(a_sb[:], aT.rearrange("(c p) b -> p c b", p=128))
            bias_sb = res_pool.tile([128, 4 * HT], F32)
            nc.sync.dma_start(bias_sb[:], bias[:])

            # [p, kchunk, gate, htile, col]
            w_r = wT.rearrange("(c p) (G t g) -> p c G t g", p=128, G=4, g=128)

            w0_sb = None
            if variant == "mm_nodma":
                w0_sb = res_pool.tile([128, KC, 4, 128], mm_dt)
                for g in range(4):
                    nc.sync.dma_start(w0_sb[:, :, g, :], w_r[:, :, g, 0, :])

            def body_mm_only():
                # PE + weight-DMA path only: no epilogue, no outputs.
                for t in range(HT):
                    if variant == "mm_nodma":
                        w_sb = w0_sb
                    else:
                        w_sb = w_pool.tile([128, KC, 4, 128], mm_dt, tag="w",
                                           name="w_sb")
                        for g in range(4):
                            nc.sync.dma_start(w_sb[:, :, g, :], w_r[:, :, g, t, :])
                    for bc in range(NBC):
                        bsl = slice(bc * BCW, (bc + 1) * BCW)
                        for g in range(4):
                            p_t = psum_pool.tile([128, BCW], F32, tag=f"ps{g}")
                            for c in range(KC):
                                nc.tensor.matmul(
                                    p_t[:], w_sb[:, c, g, :], a_sb[:, c, bsl],
                                    start=(c == 0), stop=(c == KC - 1),
                                )

            def body_ldw_reuse():
                # Same math as "full", but each stationary tile feeds the two
                # batch-chunk matmuls back to back (bc innermost).
                for t in range(HT):
                    w_sb = w_pool.tile([128, KC, 4, 128], F32R, tag="w")
                    for g in range(4):
                        nc.sync.dma_start(w_sb[:, :, g, :], w_r[:, :, g, t, :])
                    cp_sb = c_pool.tile([128, BL], F32, tag="cprev")
                    nc.sync.dma_start(cp_sb[:], cT[t * 128:(t + 1) * 128, :])
                    oc_sb = o_pool.tile([128, BL], F32, tag="oc")
                    oh_sb = o_pool.tile([128, BL], F32, tag="oh")

                    ps = {}
                    for g in range(4):
                        for bc in range(NBC):
                            ps[g, bc] = psum_pool.tile(
                                [128, BCW], F32, tag=f"ps{g}{bc}",
                                name=f"ps{g}{bc}", bufs=1)
                        for c in range(KC):
                            for bc in range(NBC):
                                bsl = slice(bc * BCW, (bc + 1) * BCW)
                                nc.tensor.matmul(
                                    ps[g, bc][:], w_sb[:, c, g, :], a_sb[:, c, bsl],
                                    start=(c == 0), stop=(c == KC - 1),
                                )
                    for bc in range(NBC):
                        bsl = slice(bc * BCW, (bc + 1) * BCW)
                        si = act_pool.tile([128, BCW], F32, tag="si")
                        sf = act_pool.tile([128, BCW], F32, tag="sf")
                        so = act_pool.tile([128, BCW], F32, tag="so")
                        tg = act_pool.tile([128, BCW], F32, tag="tg")
                        nc.scalar.activation(si[:], ps[0, bc][:], AF.Sigmoid,
                                             bias=bias_sb[:, 0 * HT + t:0 * HT + t + 1])
                        nc.scalar.activation(sf[:], ps[1, bc][:], AF.Sigmoid,
                                             bias=bias_sb[:, 1 * HT + t:1 * HT + t + 1])
                        nc.scalar.activation(so[:], ps[2, bc][:], AF.Sigmoid,
                                             bias=bias_sb[:, 2 * HT + t:2 * HT + t + 1])
                        nc.scalar.activation(tg[:], ps[3, bc][:], AF.Tanh,
                                             bias=bias_sb[:, 3 * HT + t:3 * HT + t + 1])
                        t1 = act_pool.tile([128, BCW], F32, tag="t1")
                        t2 = act_pool.tile([128, BCW], F32, tag="t2")
                        nc.vector.tensor_tensor(t1[:], sf[:], cp_sb[:, bsl], OP.mult)
                        nc.vector.tensor_tensor(t2[:], si[:], tg[:], OP.mult)
                        nc.vector.tensor_tensor(oc_sb[:, bsl], t1[:], t2[:], OP.add)
                        tct = act_pool.tile([128, BCW], F32, tag="tct")
                        nc.scalar.activation(tct[:], oc_sb[:, bsl], AF.Tanh)
                        nc.vector.tensor_tensor(oh_sb[:, bsl], so[:], tct[:], OP.mult)

                    nc.sync.dma_start(cO[t * 128:(t + 1) * 128, :], oc_sb[:])
                    nc.sync.dma_start(hO[t * 128:(t + 1) * 128, :], oh_sb[:])

            def body():
                if variant in ("mm_only", "mm_nodma", "mm_bf16"):
                    return body_mm_only()
                if variant == "ldw_reuse":
                    return body_ldw_reuse()
                for t in range(HT):
                    w_sb = w_pool.tile([128, KC, 4, 128], F32R, tag="w")
                    for g in range(4):
                        nc.sync.dma_start(w_sb[:, :, g, :], w_r[:, :, g, t, :])
                    cp_sb = c_pool.tile([128, BL], F32, tag="cprev")
                    nc.sync.dma_start(cp_sb[:], cT[t * 128:(t + 1) * 128, :])
                    oc_sb = o_pool.tile([128, BL], F32, tag="oc")
                    oh_sb = o_pool.tile([128, BL], F32, tag="oh")

                    for bc in range(NBC):
                        bsl = slice(bc * BCW, (bc + 1) * BCW)
                        ps = []
                        for g in range(4):
                            p_t = psum_pool.tile([128, BCW], F32, tag=f"ps{g}")
                            for c in range(KC):
                                nc.tensor.matmul(
                                    p_t[:], w_sb[:, c, g, :], a_sb[:, c, bsl],
                                    start=(c == 0), stop=(c == KC - 1),
                                )
                            ps.append(p_t)

                        si = act_pool.tile([128, BCW], F32, tag="si")
                        sf = act_pool.tile([128, BCW], F32, tag="sf")
                        so = act_pool.tile([128, BCW], F32, tag="so")
                        tg = act_pool.tile([128, BCW], F32, tag="tg")
                        nc.scalar.activation(si[:], ps[0][:], AF.Sigmoid,
                                             bias=bias_sb[:, 0 * HT + t:0 * HT + t + 1])
                        nc.scalar.activation(sf[:], ps[1][:], AF.Sigmoid,
                                             bias=bias_sb[:, 1 * HT + t:1 * HT + t + 1])
                        nc.scalar.activation(so[:], ps[2][:], AF.Sigmoid,
                                             bias=bias_sb[:, 2 * HT + t:2 * HT + t + 1])
                        nc.scalar.activation(tg[:], ps[3][:], AF.Tanh,
                                             bias=bias_sb[:, 3 * HT + t:3 * HT + t + 1])

                        t1 = act_pool.tile([128, BCW], F32, tag="t1")
                        t2 = act_pool.tile([128, BCW], F32, tag="t2")
                        nc.vector.tensor_tensor(t1[:], sf[:], cp_sb[:, bsl], OP.mult)
                        nc.vector.tensor_tensor(t2[:], si[:], tg[:], OP.mult)
                        nc.vector.tensor_tensor(oc_sb[:, bsl], t1[:], t2[:], OP.add)
                        tct = act_pool.tile([128, BCW], F32, tag="tct")
                        nc.scalar.activation(tct[:], oc_sb[:, bsl], AF.Tanh)
                        nc.vector.tensor_tensor(oh_sb[:, bsl], so[:], tct[:], OP.mult)

                    nc.sync.dma_start(cO[t * 128:(t + 1) * 128, :], oc_sb[:])
                    nc.sync.dma_start(hO[t * 128:(t + 1) * 128, :], oh_sb[:])

            if hw_loop and reps > 1:
                with tc.For_i(0, reps, 1,
                              hint_engines=(mybir.EngineType.PE,),
                              staggered_reset=True):
                    body()
            else:
                for _ in range(reps):
                    body()

    nc.finalize()
    return nc


def kernel(x_current, c_previous, h_previous, Wx, bx, Wh, bh):
    x = np.asarray(x_current, dtype=np.float32)
    c = np.asarray(c_previous, dtype=np.float32)
    h = np.asarray(h_previous, dtype=np.float32)
    Wx = np.asarray(Wx, dtype=np.float32)
    Wh = np.asarray(Wh, dtype=np.float32)
    bsum = np.asarray(bx, dtype=np.float32) + np.asarray(bh, dtype=np.float32)

    wT = np.ascontiguousarray(
        np.concatenate([Wx, Wh], axis=1).T)          # [2048, 4096]
    bias_t = np.ascontiguousarray(bsum.reshape(4 * HT, 128).T)  # [128, 32]

    in_maps = []
    for core in range(NCORES):
        sl = slice(core * BL, (core + 1) * BL)
        aT = np.ascontiguousarray(
            np.concatenate([x[sl], h[sl]], axis=1).T)  # [2048, BL]
        in_maps.append({
            "a_t": aT,
            "w_t": wT,
            "c_t": np.ascontiguousarray(c[sl].T),
            "bias": bias_t,
        })

    if "nc" not in _CACHE:
        _CACHE["nc"] = _build()
    nc = _CACHE["nc"]

    res = run_bass_kernel_spmd(
        nc, in_maps, list(range(NCORES)),
        trace=bool(int(os.environ.get("LSTM_TRACE", "0"))),
    )
    _CACHE["last_result"] = res

    c_out = np.empty((B, H), dtype=np.float32)
    h_out = np.empty((B, H), dtype=np.float32)
    for core in range(NCORES):
        sl = slice(core * BL, (core + 1) * BL)
        c_out[sl] = res.results[core]["c_out"].T
        h_out[sl] = res.results[core]["h_out"].T
    return c_out, h_out



# revision 4
# speedup vs baseline: 1.1148x; 1.1148x over previous
"""Fused LSTM cell on 8 Trainium2 NeuronCores.

Data-parallel over the batch: each core handles 1024 of the 8192 rows.
Per core, the two GEMMs (x @ Wx.T + h @ Wh.T) are fused into one
[2048]-contraction GEMM in bfloat16 (fp32 PSUM accumulation), with the
gate nonlinearities + state update fused into the PSUM eviction path.

Performance notes (vs the fp32r baseline at 286.5us):
- bf16 operands enable Fast Weight Load (FWL) so LDWEIGHTS hides fully
  under the 512-column matmul stream, and halve all input DMA bytes.
- Weights are pre-tiled on the host into exact consumption order
  [t, p, c, G, g] so each weight-tile DMA is one fully contiguous 2MB
  read (16KB per partition) instead of ~65K 512B strided descriptors.
- The activation panel loads per k-chunk, and the first tile's matmuls
  run c-outer, so the PE starts ~3us in instead of waiting 36us for the
  whole 8MB panel.
- Gates compute in order (g, i, f, o) with the state update interleaved,
  so after the very last matmul only sigmoid(o) * tanh(c) + one output
  DMA remain exposed.
"""

import os

import numpy as np
import ml_dtypes

import concourse.bacc as bacc
import concourse.mybir as mybir
import concourse.tile as tile
from concourse.bass_utils import run_bass_kernel_spmd

B, I, H = 8192, 1024, 1024
NCORES = 8
BL = B // NCORES        # batch rows per core
G4 = 4 * H              # stacked gate dim
KC = (I + H) // 128     # contraction chunks of 128
HT = H // 128           # h-tiles per core
NBC = 2                 # batch chunks per h-tile
BCW = BL // NBC         # 512 columns per matmul (one PSUM bank)

F32 = mybir.dt.float32
BF16 = mybir.dt.bfloat16
NP_BF16 = ml_dtypes.bfloat16
AF = mybir.ActivationFunctionType
OP = mybir.AluOpType

_CACHE: dict = {}

# Gate order in the stacked weights: (i, f, o, g). Compute g first and o
# last so the tail after the final matmul is just sigmoid(o)*tanh(c).
GATE_ORDER = (3, 0, 1, 2)


def _build():
    nc = bacc.Bacc("TRN2", target_bir_lowering=False, debug=False)
    # a_t[c, p, b] = concat(x,h).T[c*128+p, b]  (k-major, chunk-contiguous)
    aT = nc.dram_tensor("a_t", [KC, 128, BL], BF16, kind="ExternalInput")
    # w_t[t, p, c, G, g] = wT[c*128+p, G*1024+t*128+g]  (tile-contiguous)
    wT = nc.dram_tensor("w_t", [HT, 128, KC, 4, 128], BF16, kind="ExternalInput")
    cT = nc.dram_tensor("c_t", [H, BL], F32, kind="ExternalInput")
    bias = nc.dram_tensor("bias", [128, 4 * HT], F32, kind="ExternalInput")
    cO = nc.dram_tensor("c_out", [H, BL], F32, kind="ExternalOutput")
    hO = nc.dram_tensor("h_out", [H, BL], F32, kind="ExternalOutput")

    with tile.TileContext(nc) as tc:
        with (
            tc.tile_pool(name="resident", bufs=1) as res_pool,
            tc.tile_pool(name="wpool", bufs=2) as w_pool,
            tc.tile_pool(name="cpool", bufs=2) as c_pool,
            tc.tile_pool(name="opool", bufs=2) as o_pool,
            tc.tile_pool(name="act", bufs=3) as act_pool,
            tc.tile_pool(name="psum", bufs=2, space="PSUM") as psum_pool,
        ):
            # Activations resident for the whole kernel: [128, 16, 1024].
            # Chunked loads so the first matmuls only wait for chunk 0.
            a_sb = res_pool.tile([128, KC, BL], BF16)
            for c in range(KC):
                nc.sync.dma_start(a_sb[:, c, :], aT[c])
            bias_sb = res_pool.tile([128, 4 * HT], F32)
            nc.sync.dma_start(bias_sb[:], bias[:])

            def epilogue_piece(g, t, ps, cp_sb, oc_sb, oh_sb, bsl, st):
                """Emit the state-update ops that become ready once gate
                `g`'s PSUM accumulation for this (t, bc) chunk is done."""
                if g == 3:
                    st["tg"] = act_pool.tile([128, BCW], F32, tag="tg", name="tg")
                    nc.scalar.activation(st["tg"][:], ps[3][:], AF.Tanh,
                                         bias=bias_sb[:, 3 * HT + t:3 * HT + t + 1])
                elif g == 0:
                    st["si"] = act_pool.tile([128, BCW], F32, tag="si", name="si")
                    nc.scalar.activation(st["si"][:], ps[0][:], AF.Sigmoid,
                                         bias=bias_sb[:, 0 * HT + t:0 * HT + t + 1])
                    st["t2"] = act_pool.tile([128, BCW], F32, tag="t2", name="t2")
                    nc.vector.tensor_tensor(st["t2"][:], st["si"][:], st["tg"][:],
                                            OP.mult)
                elif g == 1:
                    sf = act_pool.tile([128, BCW], F32, tag="sf")
                    nc.scalar.activation(sf[:], ps[1][:], AF.Sigmoid,
                                         bias=bias_sb[:, 1 * HT + t:1 * HT + t + 1])
                    t1 = act_pool.tile([128, BCW], F32, tag="t1")
                    nc.vector.tensor_tensor(t1[:], sf[:], cp_sb[:, bsl], OP.mult)
                    nc.vector.tensor_tensor(oc_sb[:, bsl], t1[:], st["t2"][:],
                                            OP.add)
                    st["tct"] = act_pool.tile([128, BCW], F32, tag="tct", name="tct")
                    nc.scalar.activation(st["tct"][:], oc_sb[:, bsl], AF.Tanh)
                    nc.sync.dma_start(cO[t * 128:(t + 1) * 128, bsl], oc_sb[:, bsl])
                elif g == 2:
                    so = act_pool.tile([128, BCW], F32, tag="so")
                    nc.scalar.activation(so[:], ps[2][:], AF.Sigmoid,
                                         bias=bias_sb[:, 2 * HT + t:2 * HT + t + 1])
                    nc.vector.tensor_tensor(oh_sb[:, bsl], so[:], st["tct"][:],
                                            OP.mult)
                    nc.sync.dma_start(hO[t * 128:(t + 1) * 128, bsl], oh_sb[:, bsl])

            for t in range(HT):
                w_sb = w_pool.tile([128, KC, 4, 128], BF16, tag="w")
                if t == 0:
                    # First tile: c-blocked DMAs so matmuls can start as
                    # soon as the first weight block + a-chunk land.
                    CBLK = 4
                    for i in range(KC // CBLK):
                        nc.sync.dma_start(
                            w_sb[:, i * CBLK:(i + 1) * CBLK, :, :],
                            wT[t, :, i * CBLK:(i + 1) * CBLK, :, :])
                else:
                    nc.sync.dma_start(w_sb[:], wT[t])
                cp_sb = c_pool.tile([128, BL], F32, tag="cprev")
                nc.sync.dma_start(cp_sb[:], cT[t * 128:(t + 1) * 128, :])
                oc_sb = o_pool.tile([128, BL], F32, tag="oc")
                oh_sb = o_pool.tile([128, BL], F32, tag="oh")

                for bc in range(NBC):
                    bsl = slice(bc * BCW, (bc + 1) * BCW)
                    ps = {g: psum_pool.tile([128, BCW], F32, tag=f"ps{g}",
                                            name=f"ps{g}")
                          for g in GATE_ORDER}
                    st: dict = {}
                    if t == 0 and bc == 0:
                        # c-outer so each matmul only needs a-chunk c and
                        # weight block c//CBLK — PE starts ~3us in.
                        for c in range(KC):
                            for g in GATE_ORDER:
                                nc.tensor.matmul(
                                    ps[g][:], w_sb[:, c, g, :], a_sb[:, c, bsl],
                                    start=(c == 0), stop=(c == KC - 1),
                                )
                        for g in GATE_ORDER:
                            epilogue_piece(g, t, ps, cp_sb, oc_sb, oh_sb, bsl, st)
                    else:
                        for g in GATE_ORDER:
                            for c in range(KC):
                                nc.tensor.matmul(
                                    ps[g][:], w_sb[:, c, g, :], a_sb[:, c, bsl],
                                    start=(c == 0), stop=(c == KC - 1),
                                )
                            epilogue_piece(g, t, ps, cp_sb, oc_sb, oh_sb, bsl, st)

    nc.finalize()
    return nc


def kernel(x_current, c_previous, h_previous, Wx, bx, Wh, bh):
    x = np.asarray(x_current, dtype=np.float32)
    c = np.asarray(c_previous, dtype=np.float32)
    h = np.asarray(h_previous, dtype=np.float32)
    Wx = np.asarray(Wx, dtype=np.float32)
    Wh = np.asarray(Wh, dtype=np.float32)
    bsum = np.asarray(bx, dtype=np.float32) + np.asarray(bh, dtype=np.float32)

    wT = np.concatenate([Wx, Wh], axis=1).T          # [2048, 4096] fp32
    # [c, p, G, t, g] -> [t, p, c, G, g] so each tile is one contiguous 2MB
    w5 = wT.reshape(KC, 128, 4, HT, 128).transpose(3, 1, 0, 2, 4)
    w_host = np.ascontiguousarray(w5.astype(NP_BF16))
    bias_t = np.ascontiguousarray(bsum.reshape(4 * HT, 128).T)  # [128, 32]

    in_maps = []
    for core in range(NCORES):
        sl = slice(core * BL, (core + 1) * BL)
        aT = np.concatenate([x[sl], h[sl]], axis=1).T  # [2048, BL]
        a_host = np.ascontiguousarray(aT.astype(NP_BF16)).reshape(KC, 128, BL)
        in_maps.append({
            "a_t": a_host,
            "w_t": w_host,
            "c_t": np.ascontiguousarray(c[sl].T),
            "bias": bias_t,
        })

    if "nc" not in _CACHE:
        _CACHE["nc"] = _build()
    nc = _CACHE["nc"]

    res = run_bass_kernel_spmd(
        nc, in_maps, list(range(NCORES)),
        trace=bool(int(os.environ.get("LSTM_TRACE", "0"))),
    )
    _CACHE["last_result"] = res

    c_out = np.empty((B, H), dtype=np.float32)
    h_out = np.empty((B, H), dtype=np.float32)
    for core in range(NCORES):
        sl = slice(core * BL, (core + 1) * BL)
        c_out[sl] = res.results[core]["c_out"].T
        h_out[sl] = res.results[core]["h_out"].T
    return c_out, h_out


# revision 6
# speedup vs baseline: 1.1787x; 1.0573x over previous
"""Fused LSTM cell on 8 Trainium2 NeuronCores.

Data-parallel over the batch: each core handles 1024 of the 8192 rows.
Per core, the two GEMMs (x @ Wx.T + h @ Wh.T) are fused into one
[2048]-contraction GEMM in bfloat16 (fp32 PSUM accumulation), with the
gate nonlinearities + state update fused into the PSUM eviction path.

Performance notes (vs the fp32r baseline at 286.5us):
- bf16 operands enable Fast Weight Load (FWL) so LDWEIGHTS hides fully
  under the 512-column matmul stream, and halve all input DMA bytes.
- Weights are pre-tiled on the host into exact consumption order
  [t, p, c, G, g] so each weight-tile DMA is one fully contiguous 2MB
  read (16KB per partition) instead of ~65K 512B strided descriptors.
- The activation panel loads per k-chunk, and the first tile's matmuls
  run c-outer, so the PE starts ~3us in instead of waiting 36us for the
  whole 8MB panel.
- Gates compute in order (g, i, f, o) with the state update interleaved,
  so after the very last matmul only sigmoid(o) * tanh(c) + one output
  DMA remain exposed.
"""

import os

import numpy as np
import ml_dtypes

import concourse.bacc as bacc
import concourse.mybir as mybir
import concourse.tile as tile
from concourse.bass_utils import run_bass_kernel_spmd

B, I, H = 8192, 1024, 1024
NCORES = 8
BL = B // NCORES        # batch rows per core
G4 = 4 * H              # stacked gate dim
KC = (I + H) // 128     # contraction chunks of 128
HT = H // 128           # h-tiles per core
NBC = 2                 # batch chunks per h-tile
BCW = BL // NBC         # 512 columns per matmul (one PSUM bank)

F32 = mybir.dt.float32
BF16 = mybir.dt.bfloat16
NP_BF16 = ml_dtypes.bfloat16
AF = mybir.ActivationFunctionType
OP = mybir.AluOpType

_CACHE: dict = {}

# Gate order in the stacked weights: (i, f, o, g). Compute g first and o
# last so the tail after the final matmul is just sigmoid(o)*tanh(c).
GATE_ORDER = (3, 0, 1, 2)


def _build():
    nc = bacc.Bacc("TRN2", target_bir_lowering=False, debug=False)
    # a_t[c, p, b] = concat(x,h).T[c*128+p, b]  (k-major, chunk-contiguous)
    aT = nc.dram_tensor("a_t", [KC, 128, BL], BF16, kind="ExternalInput")
    # w_t[t, p, c, G, g] = wT[c*128+p, G*1024+t*128+g]  (tile-contiguous)
    wT = nc.dram_tensor("w_t", [HT, 128, KC, 4, 128], BF16, kind="ExternalInput")
    cT = nc.dram_tensor("c_t", [H, BL], F32, kind="ExternalInput")
    bias = nc.dram_tensor("bias", [128, 4 * HT], F32, kind="ExternalInput")
    cO = nc.dram_tensor("c_out", [H, BL], F32, kind="ExternalOutput")
    hO = nc.dram_tensor("h_out", [H, BL], F32, kind="ExternalOutput")

    with tile.TileContext(nc) as tc:
        with (
            tc.tile_pool(name="resident", bufs=1) as res_pool,
            tc.tile_pool(name="wpool", bufs=2) as w_pool,
            tc.tile_pool(name="cpool", bufs=2) as c_pool,
            tc.tile_pool(name="opool", bufs=2) as o_pool,
            tc.tile_pool(name="act", bufs=3) as act_pool,
            tc.tile_pool(name="psum", bufs=2, space="PSUM") as psum_pool,
        ):
            # Activations resident for the whole kernel: [128, 16, 1024].
            # All sync-ring DMAs drain in strict FIFO issue order, so
            # interleave tile-0 weight blocks with the a-chunks: the first
            # matmul only waits for w0 block 0 + a chunk 0 (~0.75MB), and
            # chunk c arrives ahead of its 8-matmul consumption slot.
            a_sb = res_pool.tile([128, KC, BL], BF16)
            w_sb0 = w_pool.tile([128, KC, 4, 128], BF16, tag="w", name="w_sb0")
            CBLK = 4
            for i in range(KC // CBLK):
                nc.sync.dma_start(
                    w_sb0[:, i * CBLK:(i + 1) * CBLK, :, :],
                    wT[0, :, i * CBLK:(i + 1) * CBLK, :, :])
                for c in range(i * CBLK, (i + 1) * CBLK):
                    nc.sync.dma_start(a_sb[:, c, :], aT[c])
            bias_sb = res_pool.tile([128, 4 * HT], F32)
            nc.sync.dma_start(bias_sb[:], bias[:])

            def epilogue_piece(g, t, ps, cp_sb, oc_sb, oh_sb, bsl, st):
                """Emit the state-update ops that become ready once gate
                `g`'s PSUM accumulation for this (t, bc) chunk is done."""
                if g == 3:
                    st["tg"] = act_pool.tile([128, BCW], F32, tag="tg", name="tg")
                    nc.scalar.activation(st["tg"][:], ps[3][:], AF.Tanh,
                                         bias=bias_sb[:, 3 * HT + t:3 * HT + t + 1])
                elif g == 0:
                    st["si"] = act_pool.tile([128, BCW], F32, tag="si", name="si")
                    nc.scalar.activation(st["si"][:], ps[0][:], AF.Sigmoid,
                                         bias=bias_sb[:, 0 * HT + t:0 * HT + t + 1])
                    st["t2"] = act_pool.tile([128, BCW], F32, tag="t2", name="t2")
                    nc.vector.tensor_tensor(st["t2"][:], st["si"][:], st["tg"][:],
                                            OP.mult)
                elif g == 1:
                    sf = act_pool.tile([128, BCW], F32, tag="sf")
                    nc.scalar.activation(sf[:], ps[1][:], AF.Sigmoid,
                                         bias=bias_sb[:, 1 * HT + t:1 * HT + t + 1])
                    t1 = act_pool.tile([128, BCW], F32, tag="t1")
                    nc.vector.tensor_tensor(t1[:], sf[:], cp_sb[:, bsl], OP.mult)
                    nc.vector.tensor_tensor(oc_sb[:, bsl], t1[:], st["t2"][:],
                                            OP.add)
                    st["tct"] = act_pool.tile([128, BCW], F32, tag="tct", name="tct")
                    nc.scalar.activation(st["tct"][:], oc_sb[:, bsl], AF.Tanh)
                    nc.sync.dma_start(cO[t * 128:(t + 1) * 128, bsl], oc_sb[:, bsl])
                elif g == 2:
                    so = act_pool.tile([128, BCW], F32, tag="so")
                    nc.scalar.activation(so[:], ps[2][:], AF.Sigmoid,
                                         bias=bias_sb[:, 2 * HT + t:2 * HT + t + 1])
                    nc.vector.tensor_tensor(oh_sb[:, bsl], so[:], st["tct"][:],
                                            OP.mult)
                    nc.sync.dma_start(hO[t * 128:(t + 1) * 128, bsl], oh_sb[:, bsl])

            for t in range(HT):
                if t == 0:
                    w_sb = w_sb0
                else:
                    w_sb = w_pool.tile([128, KC, 4, 128], BF16, tag="w")
                    nc.sync.dma_start(w_sb[:], wT[t])
                cp_sb = c_pool.tile([128, BL], F32, tag="cprev")
                nc.sync.dma_start(cp_sb[:], cT[t * 128:(t + 1) * 128, :])
                oc_sb = o_pool.tile([128, BL], F32, tag="oc")
                oh_sb = o_pool.tile([128, BL], F32, tag="oh")

                if t == 0:
                    # Tile 0: c-outer across BOTH batch chunks, so each
                    # a-chunk arrival feeds 8 matmuls (~1.7us) — faster
                    # than the ~1us DMA arrival cadence: the whole 6MB
                    # initial load hides under tile 0's matmuls.
                    pss = []
                    for bc in range(NBC):
                        pss.append({g: psum_pool.tile([128, BCW], F32,
                                                      tag=f"ps{g}",
                                                      name=f"ps{g}")
                                    for g in GATE_ORDER})
                    for c in range(KC):
                        for bc in range(NBC):
                            bsl = slice(bc * BCW, (bc + 1) * BCW)
                            for g in GATE_ORDER:
                                nc.tensor.matmul(
                                    pss[bc][g][:], w_sb[:, c, g, :],
                                    a_sb[:, c, bsl],
                                    start=(c == 0), stop=(c == KC - 1),
                                )
                    for bc in range(NBC):
                        bsl = slice(bc * BCW, (bc + 1) * BCW)
                        st: dict = {}
                        for g in GATE_ORDER:
                            epilogue_piece(g, t, pss[bc], cp_sb, oc_sb, oh_sb,
                                           bsl, st)
                    continue

                for bc in range(NBC):
                    bsl = slice(bc * BCW, (bc + 1) * BCW)
                    ps = {g: psum_pool.tile([128, BCW], F32, tag=f"ps{g}",
                                            name=f"ps{g}")
                          for g in GATE_ORDER}
                    st = {}
                    for g in GATE_ORDER:
                        for c in range(KC):
                            nc.tensor.matmul(
                                ps[g][:], w_sb[:, c, g, :], a_sb[:, c, bsl],
                                start=(c == 0), stop=(c == KC - 1),
                            )
                        epilogue_piece(g, t, ps, cp_sb, oc_sb, oh_sb, bsl, st)

    nc.finalize()
    return nc


def kernel(x_current, c_previous, h_previous, Wx, bx, Wh, bh):
    x = np.asarray(x_current, dtype=np.float32)
    c = np.asarray(c_previous, dtype=np.float32)
    h = np.asarray(h_previous, dtype=np.float32)
    Wx = np.asarray(Wx, dtype=np.float32)
    Wh = np.asarray(Wh, dtype=np.float32)
    bsum = np.asarray(bx, dtype=np.float32) + np.asarray(bh, dtype=np.float32)

    wT = np.concatenate([Wx, Wh], axis=1).T          # [2048, 4096] fp32
    # [c, p, G, t, g] -> [t, p, c, G, g] so each tile is one contiguous 2MB
    w5 = wT.reshape(KC, 128, 4, HT, 128).transpose(3, 1, 0, 2, 4)
    w_host = np.ascontiguousarray(w5.astype(NP_BF16))
    bias_t = np.ascontiguousarray(bsum.reshape(4 * HT, 128).T)  # [128, 32]

    in_maps = []
    for core in range(NCORES):
        sl = slice(core * BL, (core + 1) * BL)
        aT = np.concatenate([x[sl], h[sl]], axis=1).T  # [2048, BL]
        a_host = np.ascontiguousarray(aT.astype(NP_BF16)).reshape(KC, 128, BL)
        in_maps.append({
            "a_t": a_host,
            "w_t": w_host,
            "c_t": np.ascontiguousarray(c[sl].T),
            "bias": bias_t,
        })

    if "nc" not in _CACHE:
        _CACHE["nc"] = _build()
    nc = _CACHE["nc"]

    res = run_bass_kernel_spmd(
        nc, in_maps, list(range(NCORES)),
        trace=bool(int(os.environ.get("LSTM_TRACE", "0"))),
    )
    _CACHE["last_result"] = res

    c_out = np.empty((B, H), dtype=np.float32)
    h_out = np.empty((B, H), dtype=np.float32)
    for core in range(NCORES):
        sl = slice(core * BL, (core + 1) * BL)
        c_out[sl] = res.results[core]["c_out"].T
        h_out[sl] = res.results[core]["h_out"].T
    return c_out, h_out


# revision 8
# speedup vs baseline: 1.1860x; 1.0061x over previous
"""Fused LSTM cell on 8 Trainium2 NeuronCores.

Data-parallel over the batch: each core handles 1024 of the 8192 rows.
Per core, the two GEMMs (x @ Wx.T + h @ Wh.T) are fused into one
[2048]-contraction GEMM in bfloat16 (fp32 PSUM accumulation), with the
gate nonlinearities + state update fused into the PSUM eviction path.

Performance notes (vs the fp32r baseline at 286.5us):
- bf16 operands enable Fast Weight Load (FWL) so LDWEIGHTS hides fully
  under the 512-column matmul stream, and halve all input DMA bytes.
- Weights are pre-tiled on the host into exact consumption order
  [t, p, c, G, g] so each weight-tile DMA is one fully contiguous 2MB
  read (16KB per partition) instead of ~65K 512B strided descriptors.
- The activation panel loads per k-chunk, and the first tile's matmuls
  run c-outer, so the PE starts ~3us in instead of waiting 36us for the
  whole 8MB panel.
- Gates compute in order (g, i, f, o) with the state update interleaved,
  so after the very last matmul only sigmoid(o) * tanh(c) + one output
  DMA remain exposed.
"""

import os

import numpy as np
import ml_dtypes

import concourse.bacc as bacc
import concourse.mybir as mybir
import concourse.tile as tile
from concourse.bass_utils import run_bass_kernel_spmd

B, I, H = 8192, 1024, 1024
NCORES = 8
BL = B // NCORES        # batch rows per core
G4 = 4 * H              # stacked gate dim
KC = (I + H) // 128     # contraction chunks of 128
HT = H // 128           # h-tiles per core
NBC = 2                 # batch chunks per h-tile
BCW = BL // NBC         # 512 columns per matmul (one PSUM bank)

F32 = mybir.dt.float32
BF16 = mybir.dt.bfloat16
NP_BF16 = ml_dtypes.bfloat16
AF = mybir.ActivationFunctionType
OP = mybir.AluOpType

_CACHE: dict = {}

# Gate order in the stacked weights: (i, f, o, g). Compute g first and o
# last so the tail after the final matmul is just sigmoid(o)*tanh(c).
GATE_ORDER = (3, 0, 1, 2)


def _build():
    nc = bacc.Bacc("TRN2", target_bir_lowering=False, debug=False)
    # a_t[c, p, b] = concat(x,h).T[c*128+p, b]  (k-major, chunk-contiguous)
    aT = nc.dram_tensor("a_t", [KC, 128, BL], BF16, kind="ExternalInput")
    # w_t[t, p, c, G, g] = wT[c*128+p, G*1024+t*128+g]  (tile-contiguous)
    wT = nc.dram_tensor("w_t", [HT, 128, KC, 4, 128], BF16, kind="ExternalInput")
    cT = nc.dram_tensor("c_t", [H, BL], F32, kind="ExternalInput")
    bias = nc.dram_tensor("bias", [128, 4 * HT], F32, kind="ExternalInput")
    cO = nc.dram_tensor("c_out", [H, BL], F32, kind="ExternalOutput")
    hO = nc.dram_tensor("h_out", [H, BL], F32, kind="ExternalOutput")

    with tile.TileContext(nc) as tc:
        with (
            tc.tile_pool(name="resident", bufs=1) as res_pool,
            tc.tile_pool(name="wpool", bufs=2) as w_pool,
            tc.tile_pool(name="cpool", bufs=2) as c_pool,
            tc.tile_pool(name="opool", bufs=2) as o_pool,
            tc.tile_pool(name="act", bufs=3) as act_pool,
            tc.tile_pool(name="psum", bufs=2, space="PSUM") as psum_pool,
        ):
            # Activations resident for the whole kernel: [128, 16, 1024].
            # All sync-ring DMAs drain in strict FIFO issue order, so
            # interleave tile-0 weight blocks with the a-chunks: the first
            # matmul only waits for w0 block 0 + a chunk 0 (~0.75MB), and
            # chunk c arrives ahead of its 8-matmul consumption slot.
            a_sb = res_pool.tile([128, KC, BL], BF16)
            w_sb0 = w_pool.tile([128, KC, 4, 128], BF16, tag="w", name="w_sb0")
            # First matmul only needs w0[c=0] + a[0] (~0.4MB): issue those
            # two DMAs first, then the rest of tile 0 / the a-panel.
            nc.sync.dma_start(w_sb0[:, 0:1, :, :], wT[0, :, 0:1, :, :])
            nc.sync.dma_start(a_sb[:, 0, :], aT[0])
            nc.sync.dma_start(w_sb0[:, 1:4, :, :], wT[0, :, 1:4, :, :])
            for c in range(1, 4):
                nc.sync.dma_start(a_sb[:, c, :], aT[c])
            CBLK = 4
            for i in range(1, KC // CBLK):
                nc.sync.dma_start(
                    w_sb0[:, i * CBLK:(i + 1) * CBLK, :, :],
                    wT[0, :, i * CBLK:(i + 1) * CBLK, :, :])
                for c in range(i * CBLK, (i + 1) * CBLK):
                    nc.sync.dma_start(a_sb[:, c, :], aT[c])
            bias_sb = res_pool.tile([128, 4 * HT], F32)
            nc.sync.dma_start(bias_sb[:], bias[:])
            # PE warm-up: stream garbage matmuls while the first DMAs are
            # in flight so HAM reaches the warm 2.4GHz state before the
            # real accumulation starts. No DMA dependency (memset inputs);
            # results land in tile 0's first PSUM tile and are overwritten
            # by the real start=True matmul.
            warm_w = res_pool.tile([128, 128], BF16)
            warm_a = res_pool.tile([128, BCW], BF16)
            nc.vector.memset(warm_w[:], 0)
            nc.vector.memset(warm_a[:], 0)

            def epilogue_piece(g, t, ps, cp_sb, oc_sb, oh_sb, bsl, st):
                """Emit the state-update ops that become ready once gate
                `g`'s PSUM accumulation for this (t, bc) chunk is done."""
                if g == 3:
                    st["tg"] = act_pool.tile([128, BCW], F32, tag="tg", name="tg")
                    nc.scalar.activation(st["tg"][:], ps[3][:], AF.Tanh,
                                         bias=bias_sb[:, 3 * HT + t:3 * HT + t + 1])
                elif g == 0:
                    st["si"] = act_pool.tile([128, BCW], F32, tag="si", name="si")
                    nc.scalar.activation(st["si"][:], ps[0][:], AF.Sigmoid,
                                         bias=bias_sb[:, 0 * HT + t:0 * HT + t + 1])
                    st["t2"] = act_pool.tile([128, BCW], F32, tag="t2", name="t2")
                    nc.vector.tensor_tensor(st["t2"][:], st["si"][:], st["tg"][:],
                                            OP.mult)
                elif g == 1:
                    sf = act_pool.tile([128, BCW], F32, tag="sf")
                    nc.scalar.activation(sf[:], ps[1][:], AF.Sigmoid,
                                         bias=bias_sb[:, 1 * HT + t:1 * HT + t + 1])
                    t1 = act_pool.tile([128, BCW], F32, tag="t1")
                    nc.vector.tensor_tensor(t1[:], sf[:], cp_sb[:, bsl], OP.mult)
                    nc.vector.tensor_tensor(oc_sb[:, bsl], t1[:], st["t2"][:],
                                            OP.add)
                    st["tct"] = act_pool.tile([128, BCW], F32, tag="tct", name="tct")
                    nc.scalar.activation(st["tct"][:], oc_sb[:, bsl], AF.Tanh)
                    nc.sync.dma_start(cO[t * 128:(t + 1) * 128, bsl], oc_sb[:, bsl])
                elif g == 2:
                    so = act_pool.tile([128, BCW], F32, tag="so")
                    nc.scalar.activation(so[:], ps[2][:], AF.Sigmoid,
                                         bias=bias_sb[:, 2 * HT + t:2 * HT + t + 1])
                    nc.vector.tensor_tensor(oh_sb[:, bsl], so[:], st["tct"][:],
                                            OP.mult)
                    nc.sync.dma_start(hO[t * 128:(t + 1) * 128, bsl], oh_sb[:, bsl])

            for t in range(HT):
                if t == 0:
                    w_sb = w_sb0
                else:
                    w_sb = w_pool.tile([128, KC, 4, 128], BF16, tag="w")
                    nc.sync.dma_start(w_sb[:], wT[t])
                cp_sb = c_pool.tile([128, BL], F32, tag="cprev")
                nc.sync.dma_start(cp_sb[:], cT[t * 128:(t + 1) * 128, :])
                oc_sb = o_pool.tile([128, BL], F32, tag="oc")
                oh_sb = o_pool.tile([128, BL], F32, tag="oh")

                if t == 0:
                    # Tile 0: c-outer across BOTH batch chunks, so each
                    # a-chunk arrival feeds 8 matmuls (~1.7us) — faster
                    # than the ~1us DMA arrival cadence: the whole 6MB
                    # initial load hides under tile 0's matmuls.
                    pss = []
                    for bc in range(NBC):
                        pss.append({g: psum_pool.tile([128, BCW], F32,
                                                      tag=f"ps{g}",
                                                      name=f"ps{g}")
                                    for g in GATE_ORDER})
                    for _ in range(24):
                        nc.tensor.matmul(pss[0][GATE_ORDER[0]][:],
                                         warm_w[:], warm_a[:],
                                         start=True, stop=True)
                    for c in range(KC):
                        for bc in range(NBC):
                            bsl = slice(bc * BCW, (bc + 1) * BCW)
                            for g in GATE_ORDER:
                                nc.tensor.matmul(
                                    pss[bc][g][:], w_sb[:, c, g, :],
                                    a_sb[:, c, bsl],
                                    start=(c == 0), stop=(c == KC - 1),
                                )
                    for bc in range(NBC):
                        bsl = slice(bc * BCW, (bc + 1) * BCW)
                        st: dict = {}
                        for g in GATE_ORDER:
                            epilogue_piece(g, t, pss[bc], cp_sb, oc_sb, oh_sb,
                                           bsl, st)
                    continue

                for bc in range(NBC):
                    bsl = slice(bc * BCW, (bc + 1) * BCW)
                    ps = {g: psum_pool.tile([128, BCW], F32, tag=f"ps{g}",
                                            name=f"ps{g}")
                          for g in GATE_ORDER}
                    st = {}
                    for g in GATE_ORDER:
                        for c in range(KC):
                            nc.tensor.matmul(
                                ps[g][:], w_sb[:, c, g, :], a_sb[:, c, bsl],
                                start=(c == 0), stop=(c == KC - 1),
                            )
                        epilogue_piece(g, t, ps, cp_sb, oc_sb, oh_sb, bsl, st)

    nc.finalize()
    return nc


def kernel(x_current, c_previous, h_previous, Wx, bx, Wh, bh):
    x = np.asarray(x_current, dtype=np.float32)
    c = np.asarray(c_previous, dtype=np.float32)
    h = np.asarray(h_previous, dtype=np.float32)
    Wx = np.asarray(Wx, dtype=np.float32)
    Wh = np.asarray(Wh, dtype=np.float32)
    bsum = np.asarray(bx, dtype=np.float32) + np.asarray(bh, dtype=np.float32)

    wT = np.concatenate([Wx, Wh], axis=1).T          # [2048, 4096] fp32
    # [c, p, G, t, g] -> [t, p, c, G, g] so each tile is one contiguous 2MB
    w5 = wT.reshape(KC, 128, 4, HT, 128).transpose(3, 1, 0, 2, 4)
    w_host = np.ascontiguousarray(w5.astype(NP_BF16))
    bias_t = np.ascontiguousarray(bsum.reshape(4 * HT, 128).T)  # [128, 32]

    in_maps = []
    for core in range(NCORES):
        sl = slice(core * BL, (core + 1) * BL)
        aT = np.concatenate([x[sl], h[sl]], axis=1).T  # [2048, BL]
        a_host = np.ascontiguousarray(aT.astype(NP_BF16)).reshape(KC, 128, BL)
        in_maps.append({
            "a_t": a_host,
            "w_t": w_host,
            "c_t": np.ascontiguousarray(c[sl].T),
            "bias": bias_t,
        })

    if "nc" not in _CACHE:
        _CACHE["nc"] = _build()
    nc = _CACHE["nc"]

    res = run_bass_kernel_spmd(
        nc, in_maps, list(range(NCORES)),
        trace=bool(int(os.environ.get("LSTM_TRACE", "0"))),
    )
    _CACHE["last_result"] = res

    c_out = np.empty((B, H), dtype=np.float32)
    h_out = np.empty((B, H), dtype=np.float32)
    for core in range(NCORES):
        sl = slice(core * BL, (core + 1) * BL)
        c_out[sl] = res.results[core]["c_out"].T
        h_out[sl] = res.results[core]["h_out"].T
    return c_out, h_out


# revision 9
# speedup vs baseline: 1.1872x; 1.0011x over previous
"""Fused LSTM cell on 8 Trainium2 NeuronCores.

Data-parallel over the batch: each core handles 1024 of the 8192 rows.
Per core, the two GEMMs (x @ Wx.T + h @ Wh.T) are fused into one
[2048]-contraction GEMM in bfloat16 (fp32 PSUM accumulation), with the
gate nonlinearities + state update fused into the PSUM eviction path.

Performance notes (vs the fp32r baseline at 286.5us):
- bf16 operands enable Fast Weight Load (FWL) so LDWEIGHTS hides fully
  under the 512-column matmul stream, and halve all input DMA bytes.
- Weights are pre-tiled on the host into exact consumption order
  [t, p, c, G, g] so each weight-tile DMA is one fully contiguous 2MB
  read (16KB per partition) instead of ~65K 512B strided descriptors.
- The activation panel loads per k-chunk, and the first tile's matmuls
  run c-outer, so the PE starts ~3us in instead of waiting 36us for the
  whole 8MB panel.
- Gates compute in order (g, i, f, o) with the state update interleaved,
  so after the very last matmul only sigmoid(o) * tanh(c) + one output
  DMA remain exposed.
"""

import os

import numpy as np
import ml_dtypes

import concourse.bacc as bacc
import concourse.mybir as mybir
import concourse.tile as tile
from concourse.bass_utils import run_bass_kernel_spmd

B, I, H = 8192, 1024, 1024
NCORES = 8
BL = B // NCORES        # batch rows per core
G4 = 4 * H              # stacked gate dim
KC = (I + H) // 128     # contraction chunks of 128
HT = H // 128           # h-tiles per core
NBC = 2                 # batch chunks per h-tile
BCW = BL // NBC         # 512 columns per matmul (one PSUM bank)

F32 = mybir.dt.float32
BF16 = mybir.dt.bfloat16
NP_BF16 = ml_dtypes.bfloat16
AF = mybir.ActivationFunctionType
OP = mybir.AluOpType

_CACHE: dict = {}

# Gate order in the stacked weights: (i, f, o, g). Compute g first and o
# last so the tail after the final matmul is just sigmoid(o)*tanh(c).
GATE_ORDER = (3, 0, 1, 2)


def _build():
    nc = bacc.Bacc("TRN2", target_bir_lowering=False, debug=False)
    # a_t[c, p, b] = concat(x,h).T[c*128+p, b]  (k-major, chunk-contiguous)
    aT = nc.dram_tensor("a_t", [KC, 128, BL], BF16, kind="ExternalInput")
    # w_t[t, p, c, G, g] = wT[c*128+p, G*1024+t*128+g]  (tile-contiguous)
    wT = nc.dram_tensor("w_t", [HT, 128, KC, 4, 128], BF16, kind="ExternalInput")
    cT = nc.dram_tensor("c_t", [H, BL], F32, kind="ExternalInput")
    bias = nc.dram_tensor("bias", [128, 4 * HT], F32, kind="ExternalInput")
    cO = nc.dram_tensor("c_out", [H, BL], F32, kind="ExternalOutput")
    hO = nc.dram_tensor("h_out", [H, BL], F32, kind="ExternalOutput")

    with tile.TileContext(nc) as tc:
        with (
            tc.tile_pool(name="resident", bufs=1) as res_pool,
            tc.tile_pool(name="wpool", bufs=2) as w_pool,
            tc.tile_pool(name="cpool", bufs=2) as c_pool,
            tc.tile_pool(name="opool", bufs=2) as o_pool,
            tc.tile_pool(name="act", bufs=3) as act_pool,
            tc.tile_pool(name="psum", bufs=2, space="PSUM") as psum_pool,
        ):
            # Activations resident for the whole kernel: [128, 16, 1024].
            # All sync-ring DMAs drain in strict FIFO issue order, so
            # interleave tile-0 weight blocks with the a-chunks: the first
            # matmul only waits for w0 block 0 + a chunk 0 (~0.75MB), and
            # chunk c arrives ahead of its 8-matmul consumption slot.
            a_sb = res_pool.tile([128, KC, BL], BF16)
            w_sb0 = w_pool.tile([128, KC, 4, 128], BF16, tag="w", name="w_sb0")
            # First matmul only needs w0[c=0] + a[0] (~0.4MB): issue those
            # two DMAs first, then the rest of tile 0 / the a-panel.
            nc.sync.dma_start(w_sb0[:, 0:1, :, :], wT[0, :, 0:1, :, :])
            nc.sync.dma_start(a_sb[:, 0, :], aT[0])
            nc.sync.dma_start(w_sb0[:, 1:4, :, :], wT[0, :, 1:4, :, :])
            for c in range(1, 4):
                nc.sync.dma_start(a_sb[:, c, :], aT[c])
            CBLK = 4
            for i in range(1, KC // CBLK):
                nc.sync.dma_start(
                    w_sb0[:, i * CBLK:(i + 1) * CBLK, :, :],
                    wT[0, :, i * CBLK:(i + 1) * CBLK, :, :])
                for c in range(i * CBLK, (i + 1) * CBLK):
                    nc.sync.dma_start(a_sb[:, c, :], aT[c])
            bias_sb = res_pool.tile([128, 4 * HT], F32)
            nc.sync.dma_start(bias_sb[:], bias[:])
            # PE warm-up: stream garbage matmuls while the first DMAs are
            # in flight so HAM reaches the warm 2.4GHz state before the
            # real accumulation starts. No DMA dependency (memset inputs);
            # results land in tile 0's first PSUM tile and are overwritten
            # by the real start=True matmul.
            warm_w = res_pool.tile([128, 128], BF16)
            warm_a = res_pool.tile([128, BCW], BF16)
            nc.vector.memset(warm_w[:], 0)
            nc.vector.memset(warm_a[:], 0)

            def epilogue_piece(g, t, ps, cp_sb, oc_sb, oh_sb, bsl, st):
                """Emit the state-update ops that become ready once gate
                `g`'s PSUM accumulation for this (t, bc) chunk is done."""
                if g == 3:
                    st["tg"] = act_pool.tile([128, BCW], F32, tag="tg", name="tg")
                    nc.scalar.activation(st["tg"][:], ps[3][:], AF.Tanh,
                                         bias=bias_sb[:, 3 * HT + t:3 * HT + t + 1])
                elif g == 0:
                    st["si"] = act_pool.tile([128, BCW], F32, tag="si", name="si")
                    nc.scalar.activation(st["si"][:], ps[0][:], AF.Sigmoid,
                                         bias=bias_sb[:, 0 * HT + t:0 * HT + t + 1])
                    st["t2"] = act_pool.tile([128, BCW], F32, tag="t2", name="t2")
                    nc.vector.tensor_tensor(st["t2"][:], st["si"][:], st["tg"][:],
                                            OP.mult)
                elif g == 1:
                    sf = act_pool.tile([128, BCW], F32, tag="sf")
                    nc.scalar.activation(sf[:], ps[1][:], AF.Sigmoid,
                                         bias=bias_sb[:, 1 * HT + t:1 * HT + t + 1])
                    t1 = act_pool.tile([128, BCW], F32, tag="t1")
                    nc.vector.tensor_tensor(t1[:], sf[:], cp_sb[:, bsl], OP.mult)
                    nc.vector.tensor_tensor(oc_sb[:, bsl], t1[:], st["t2"][:],
                                            OP.add)
                    st["tct"] = act_pool.tile([128, BCW], F32, tag="tct", name="tct")
                    nc.scalar.activation(st["tct"][:], oc_sb[:, bsl], AF.Tanh)
                    nc.sync.dma_start(cO[t * 128:(t + 1) * 128, bsl], oc_sb[:, bsl])
                elif g == 2:
                    so = act_pool.tile([128, BCW], F32, tag="so")
                    nc.scalar.activation(so[:], ps[2][:], AF.Sigmoid,
                                         bias=bias_sb[:, 2 * HT + t:2 * HT + t + 1])
                    nc.vector.tensor_tensor(oh_sb[:, bsl], so[:], st["tct"][:],
                                            OP.mult)
                    nc.sync.dma_start(hO[t * 128:(t + 1) * 128, bsl], oh_sb[:, bsl])

            for t in range(HT):
                if t == 0:
                    w_sb = w_sb0
                else:
                    w_sb = w_pool.tile([128, KC, 4, 128], BF16, tag="w")
                    nc.sync.dma_start(w_sb[:], wT[t])
                cp_sb = c_pool.tile([128, BL], F32, tag="cprev")
                nc.sync.dma_start(cp_sb[:], cT[t * 128:(t + 1) * 128, :])
                oc_sb = o_pool.tile([128, BL], F32, tag="oc")
                oh_sb = o_pool.tile([128, BL], F32, tag="oh")

                if t == 0:
                    # Tile 0: c-outer across BOTH batch chunks, so each
                    # a-chunk arrival feeds 8 matmuls (~1.7us) — faster
                    # than the ~1us DMA arrival cadence: the whole 6MB
                    # initial load hides under tile 0's matmuls.
                    pss = []
                    for bc in range(NBC):
                        pss.append({g: psum_pool.tile([128, BCW], F32,
                                                      tag=f"ps{g}",
                                                      name=f"ps{g}")
                                    for g in GATE_ORDER})
                    for _ in range(14):
                        nc.tensor.matmul(pss[0][GATE_ORDER[0]][:],
                                         warm_w[:], warm_a[:],
                                         start=True, stop=True)
                    for c in range(KC):
                        for bc in range(NBC):
                            bsl = slice(bc * BCW, (bc + 1) * BCW)
                            for g in GATE_ORDER:
                                nc.tensor.matmul(
                                    pss[bc][g][:], w_sb[:, c, g, :],
                                    a_sb[:, c, bsl],
                                    start=(c == 0), stop=(c == KC - 1),
                                )
                    for bc in range(NBC):
                        bsl = slice(bc * BCW, (bc + 1) * BCW)
                        st: dict = {}
                        for g in GATE_ORDER:
                            epilogue_piece(g, t, pss[bc], cp_sb, oc_sb, oh_sb,
                                           bsl, st)
                    continue

                for bc in range(NBC):
                    bsl = slice(bc * BCW, (bc + 1) * BCW)
                    ps = {g: psum_pool.tile([128, BCW], F32, tag=f"ps{g}",
                                            name=f"ps{g}")
                          for g in GATE_ORDER}
                    st = {}
                    for g in GATE_ORDER:
                        for c in range(KC):
                            nc.tensor.matmul(
                                ps[g][:], w_sb[:, c, g, :], a_sb[:, c, bsl],
                                start=(c == 0), stop=(c == KC - 1),
                            )
                        epilogue_piece(g, t, ps, cp_sb, oc_sb, oh_sb, bsl, st)

    nc.finalize()
    return nc


def kernel(x_current, c_previous, h_previous, Wx, bx, Wh, bh):
    x = np.asarray(x_current, dtype=np.float32)
    c = np.asarray(c_previous, dtype=np.float32)
    h = np.asarray(h_previous, dtype=np.float32)
    Wx = np.asarray(Wx, dtype=np.float32)
    Wh = np.asarray(Wh, dtype=np.float32)
    bsum = np.asarray(bx, dtype=np.float32) + np.asarray(bh, dtype=np.float32)

    wT = np.concatenate([Wx, Wh], axis=1).T          # [2048, 4096] fp32
    # [c, p, G, t, g] -> [t, p, c, G, g] so each tile is one contiguous 2MB
    w5 = wT.reshape(KC, 128, 4, HT, 128).transpose(3, 1, 0, 2, 4)
    w_host = np.ascontiguousarray(w5.astype(NP_BF16))
    bias_t = np.ascontiguousarray(bsum.reshape(4 * HT, 128).T)  # [128, 32]

    in_maps = []
    for core in range(NCORES):
        sl = slice(core * BL, (core + 1) * BL)
        aT = np.concatenate([x[sl], h[sl]], axis=1).T  # [2048, BL]
        a_host = np.ascontiguousarray(aT.astype(NP_BF16)).reshape(KC, 128, BL)
        in_maps.append({
            "a_t": a_host,
            "w_t": w_host,
            "c_t": np.ascontiguousarray(c[sl].T),
            "bias": bias_t,
        })

    if "nc" not in _CACHE:
        _CACHE["nc"] = _build()
    nc = _CACHE["nc"]

    res = run_bass_kernel_spmd(
        nc, in_maps, list(range(NCORES)),
        trace=bool(int(os.environ.get("LSTM_TRACE", "0"))),
    )
    _CACHE["last_result"] = res

    c_out = np.empty((B, H), dtype=np.float32)
    h_out = np.empty((B, H), dtype=np.float32)
    for core in range(NCORES):
        sl = slice(core * BL, (core + 1) * BL)
        c_out[sl] = res.results[core]["c_out"].T
        h_out[sl] = res.results[core]["h_out"].T
    return c_out, h_out


# revision 10
# speedup vs baseline: 1.1940x; 1.0057x over previous
"""Fused LSTM cell on 8 Trainium2 NeuronCores.

Data-parallel over the batch: each core handles 1024 of the 8192 rows.
Per core, the two GEMMs (x @ Wx.T + h @ Wh.T) are fused into one
[2048]-contraction GEMM in bfloat16 (fp32 PSUM accumulation), with the
gate nonlinearities + state update fused into the PSUM eviction path.

Performance notes (vs the fp32r baseline at 286.5us):
- bf16 operands enable Fast Weight Load (FWL) so LDWEIGHTS hides fully
  under the 512-column matmul stream, and halve all input DMA bytes.
- Weights are pre-tiled on the host into exact consumption order
  [t, p, c, G, g] so each weight-tile DMA is one fully contiguous 2MB
  read (16KB per partition) instead of ~65K 512B strided descriptors.
- The activation panel loads per k-chunk, and the first tile's matmuls
  run c-outer, so the PE starts ~3us in instead of waiting 36us for the
  whole 8MB panel.
- Gates compute in order (g, i, f, o) with the state update interleaved,
  so after the very last matmul only sigmoid(o) * tanh(c) + one output
  DMA remain exposed.
"""

import os

import numpy as np
import ml_dtypes

import concourse.bacc as bacc
import concourse.mybir as mybir
import concourse.tile as tile
from concourse.bass_utils import run_bass_kernel_spmd

B, I, H = 8192, 1024, 1024
NCORES = 8
BL = B // NCORES        # batch rows per core
G4 = 4 * H              # stacked gate dim
KC = (I + H) // 128     # contraction chunks of 128
HT = H // 128           # h-tiles per core
NBC = 2                 # batch chunks per h-tile
BCW = BL // NBC         # 512 columns per matmul (one PSUM bank)

F32 = mybir.dt.float32
BF16 = mybir.dt.bfloat16
NP_BF16 = ml_dtypes.bfloat16
AF = mybir.ActivationFunctionType
OP = mybir.AluOpType

_CACHE: dict = {}

# Gate order in the stacked weights: (i, f, o, g). Compute g first and o
# last so the tail after the final matmul is just sigmoid(o)*tanh(c).
GATE_ORDER = (3, 0, 1, 2)


def _build():
    nc = bacc.Bacc("TRN2", target_bir_lowering=False, debug=False)
    # a_t[c, p, b] = concat(x,h).T[c*128+p, b]  (k-major, chunk-contiguous)
    aT = nc.dram_tensor("a_t", [KC, 128, BL], BF16, kind="ExternalInput")
    # w_t[t, p, c, G, g] = wT[c*128+p, G*1024+t*128+g]  (tile-contiguous)
    wT = nc.dram_tensor("w_t", [HT, 128, KC, 4, 128], BF16, kind="ExternalInput")
    cT = nc.dram_tensor("c_t", [H, BL], F32, kind="ExternalInput")
    bias = nc.dram_tensor("bias", [128, 4 * HT], F32, kind="ExternalInput")
    cO = nc.dram_tensor("c_out", [H, BL], F32, kind="ExternalOutput")
    hO = nc.dram_tensor("h_out", [H, BL], F32, kind="ExternalOutput")

    with tile.TileContext(nc) as tc:
        with (
            tc.tile_pool(name="resident", bufs=1) as res_pool,
            tc.tile_pool(name="wpool", bufs=2) as w_pool,
            tc.tile_pool(name="cpool", bufs=2) as c_pool,
            tc.tile_pool(name="opool", bufs=2) as o_pool,
            tc.tile_pool(name="act", bufs=3) as act_pool,
            tc.tile_pool(name="psum", bufs=2, space="PSUM") as psum_pool,
        ):
            # Activations resident for the whole kernel: [128, 16, 1024].
            # All sync-ring DMAs drain in strict FIFO issue order, so
            # interleave tile-0 weight blocks with the a-chunks: the first
            # matmul only waits for w0 block 0 + a chunk 0 (~0.75MB), and
            # chunk c arrives ahead of its 8-matmul consumption slot.
            a_sb = res_pool.tile([128, KC, BL], BF16)
            w_sb0 = w_pool.tile([128, KC, 4, 128], BF16, tag="w", name="w_sb0")
            # First matmul only needs w0[c=0] + a[0] (~0.4MB): issue those
            # two DMAs first, then the rest of tile 0 / the a-panel.
            nc.sync.dma_start(w_sb0[:, 0:1, :, :], wT[0, :, 0:1, :, :])
            nc.sync.dma_start(a_sb[:, 0, :], aT[0])
            nc.sync.dma_start(w_sb0[:, 1:4, :, :], wT[0, :, 1:4, :, :])
            for c in range(1, 4):
                nc.sync.dma_start(a_sb[:, c, :], aT[c])
            CBLK = 4
            for i in range(1, KC // CBLK):
                nc.sync.dma_start(
                    w_sb0[:, i * CBLK:(i + 1) * CBLK, :, :],
                    wT[0, :, i * CBLK:(i + 1) * CBLK, :, :])
                for c in range(i * CBLK, (i + 1) * CBLK):
                    nc.sync.dma_start(a_sb[:, c, :], aT[c])
            bias_sb = res_pool.tile([128, 4 * HT], F32)
            nc.sync.dma_start(bias_sb[:], bias[:])
            # PE warm-up: stream garbage matmuls while the first DMAs are
            # in flight so HAM reaches the warm 2.4GHz state before the
            # real accumulation starts. No DMA dependency (memset inputs);
            # results land in tile 0's first PSUM tile and are overwritten
            # by the real start=True matmul.
            warm_w = res_pool.tile([128, 128], BF16)
            warm_a = res_pool.tile([128, BCW], BF16)
            nc.vector.memset(warm_w[:], 0)
            nc.vector.memset(warm_a[:], 0)

            def epilogue_piece(g, t, ps, cp_sb, oc_sb, oh_sb, bsl, st):
                """Emit the state-update ops that become ready once gate
                `g`'s PSUM accumulation for this (t, bc) chunk is done."""
                if g == 3:
                    st["tg"] = act_pool.tile([128, BCW], F32, tag="tg", name="tg")
                    nc.scalar.activation(st["tg"][:], ps[3][:], AF.Tanh,
                                         bias=bias_sb[:, 3 * HT + t:3 * HT + t + 1])
                elif g == 0:
                    st["si"] = act_pool.tile([128, BCW], F32, tag="si", name="si")
                    nc.scalar.activation(st["si"][:], ps[0][:], AF.Sigmoid,
                                         bias=bias_sb[:, 0 * HT + t:0 * HT + t + 1])
                    st["t2"] = act_pool.tile([128, BCW], F32, tag="t2", name="t2")
                    nc.vector.tensor_tensor(st["t2"][:], st["si"][:], st["tg"][:],
                                            OP.mult)
                elif g == 1:
                    sf = act_pool.tile([128, BCW], F32, tag="sf")
                    nc.scalar.activation(sf[:], ps[1][:], AF.Sigmoid,
                                         bias=bias_sb[:, 1 * HT + t:1 * HT + t + 1])
                    t1 = act_pool.tile([128, BCW], F32, tag="t1")
                    nc.vector.tensor_tensor(t1[:], sf[:], cp_sb[:, bsl], OP.mult)
                    nc.vector.tensor_tensor(oc_sb[:, bsl], t1[:], st["t2"][:],
                                            OP.add)
                    st["tct"] = act_pool.tile([128, BCW], F32, tag="tct", name="tct")
                    nc.scalar.activation(st["tct"][:], oc_sb[:, bsl], AF.Tanh)
                    nc.scalar.dma_start(cO[t * 128:(t + 1) * 128, bsl], oc_sb[:, bsl])
                elif g == 2:
                    so = act_pool.tile([128, BCW], F32, tag="so")
                    nc.scalar.activation(so[:], ps[2][:], AF.Sigmoid,
                                         bias=bias_sb[:, 2 * HT + t:2 * HT + t + 1])
                    nc.vector.tensor_tensor(oh_sb[:, bsl], so[:], st["tct"][:],
                                            OP.mult)
                    nc.scalar.dma_start(hO[t * 128:(t + 1) * 128, bsl], oh_sb[:, bsl])

            for t in range(HT):
                if t == 0:
                    w_sb = w_sb0
                else:
                    w_sb = w_pool.tile([128, KC, 4, 128], BF16, tag="w")
                    nc.sync.dma_start(w_sb[:], wT[t])
                cp_sb = c_pool.tile([128, BL], F32, tag="cprev")
                nc.sync.dma_start(cp_sb[:], cT[t * 128:(t + 1) * 128, :])
                oc_sb = o_pool.tile([128, BL], F32, tag="oc")
                oh_sb = o_pool.tile([128, BL], F32, tag="oh")

                if t == 0:
                    # Tile 0: c-outer across BOTH batch chunks, so each
                    # a-chunk arrival feeds 8 matmuls (~1.7us) — faster
                    # than the ~1us DMA arrival cadence: the whole 6MB
                    # initial load hides under tile 0's matmuls.
                    pss = []
                    for bc in range(NBC):
                        pss.append({g: psum_pool.tile([128, BCW], F32,
                                                      tag=f"ps{g}",
                                                      name=f"ps{g}")
                                    for g in GATE_ORDER})
                    for _ in range(14):
                        nc.tensor.matmul(pss[0][GATE_ORDER[0]][:],
                                         warm_w[:], warm_a[:],
                                         start=True, stop=True)
                    for c in range(KC):
                        for bc in range(NBC):
                            bsl = slice(bc * BCW, (bc + 1) * BCW)
                            for g in GATE_ORDER:
                                nc.tensor.matmul(
                                    pss[bc][g][:], w_sb[:, c, g, :],
                                    a_sb[:, c, bsl],
                                    start=(c == 0), stop=(c == KC - 1),
                                )
                    for bc in range(NBC):
                        bsl = slice(bc * BCW, (bc + 1) * BCW)
                        st: dict = {}
                        for g in GATE_ORDER:
                            epilogue_piece(g, t, pss[bc], cp_sb, oc_sb, oh_sb,
                                           bsl, st)
                    continue

                for bc in range(NBC):
                    bsl = slice(bc * BCW, (bc + 1) * BCW)
                    ps = {g: psum_pool.tile([128, BCW], F32, tag=f"ps{g}",
                                            name=f"ps{g}")
                          for g in GATE_ORDER}
                    st = {}
                    for g in GATE_ORDER:
                        for c in range(KC):
                            nc.tensor.matmul(
                                ps[g][:], w_sb[:, c, g, :], a_sb[:, c, bsl],
                                start=(c == 0), stop=(c == KC - 1),
                            )
                        epilogue_piece(g, t, ps, cp_sb, oc_sb, oh_sb, bsl, st)

    nc.finalize()
    return nc


def kernel(x_current, c_previous, h_previous, Wx, bx, Wh, bh):
    x = np.asarray(x_current, dtype=np.float32)
    c = np.asarray(c_previous, dtype=np.float32)
    h = np.asarray(h_previous, dtype=np.float32)
    Wx = np.asarray(Wx, dtype=np.float32)
    Wh = np.asarray(Wh, dtype=np.float32)
    bsum = np.asarray(bx, dtype=np.float32) + np.asarray(bh, dtype=np.float32)

    wT = np.concatenate([Wx, Wh], axis=1).T          # [2048, 4096] fp32
    # [c, p, G, t, g] -> [t, p, c, G, g] so each tile is one contiguous 2MB
    w5 = wT.reshape(KC, 128, 4, HT, 128).transpose(3, 1, 0, 2, 4)
    w_host = np.ascontiguousarray(w5.astype(NP_BF16))
    bias_t = np.ascontiguousarray(bsum.reshape(4 * HT, 128).T)  # [128, 32]

    in_maps = []
    for core in range(NCORES):
        sl = slice(core * BL, (core + 1) * BL)
        aT = np.concatenate([x[sl], h[sl]], axis=1).T  # [2048, BL]
        a_host = np.ascontiguousarray(aT.astype(NP_BF16)).reshape(KC, 128, BL)
        in_maps.append({
            "a_t": a_host,
            "w_t": w_host,
            "c_t": np.ascontiguousarray(c[sl].T),
            "bias": bias_t,
        })

    if "nc" not in _CACHE:
        _CACHE["nc"] = _build()
    nc = _CACHE["nc"]

    res = run_bass_kernel_spmd(
        nc, in_maps, list(range(NCORES)),
        trace=bool(int(os.environ.get("LSTM_TRACE", "0"))),
    )
    _CACHE["last_result"] = res

    c_out = np.empty((B, H), dtype=np.float32)
    h_out = np.empty((B, H), dtype=np.float32)
    for core in range(NCORES):
        sl = slice(core * BL, (core + 1) * BL)
        c_out[sl] = res.results[core]["c_out"].T
        h_out[sl] = res.results[core]["h_out"].T
    return c_out, h_out
